# revision 1
# baseline (speedup 1.0000x reference)
"""NeRF renderer kernel for 8 Trainium2 NeuronCores.

Contract: kernel(**inputs) takes FULL unsharded inputs (rays_o [32768,3],
rays_d [32768,3], MLP params, num_steps=128) and returns the FULL [32768,9]
output. Rays are sharded 8 ways (4096 rays/core); params are replicated.
Each core runs a Bass kernel dispatched via run_bass_kernel_spmd.
"""

import sys

for _p in ("/opt/trn_rl_repo", "/root/.axon_site/_ro/trn_rl_repo"):
    if _p not in sys.path:
        sys.path.insert(0, _p)

import numpy as np

N_CORES = 8
N_RAYS = 32768
T = 128
BOUND = 1.0
MIN_NEAR = 0.2
EPS = 1e-15

_CACHED = {}


def _build_copy_module(n_rays_per_core: int):
    """Bass module: per-core [n,9] ray-result passthrough (DRAM->DRAM DMA)."""
    import concourse.bass as bass
    import concourse.mybir as mybir

    nc = bass.Bass(target_bir_lowering=False, debug=False)
    x = nc.dram_tensor(
        "x", [n_rays_per_core, 9], mybir.dt.float32, kind="ExternalInput"
    )
    y = nc.dram_tensor(
        "y", [n_rays_per_core, 9], mybir.dt.float32, kind="ExternalOutput"
    )
    with nc.Block() as block, nc.semaphore("dma_sem") as dma_sem:

        @block.gpsimd
        def _(gpsimd):
            gpsimd.dma_start(out=y[:], in_=x[:]).then_inc(dma_sem, 16)
            gpsimd.wait_ge(dma_sem, 16)

    return nc


def _host_reference_chunk(rays_o, rays_d, W1, b1, Wsig, Wsig_d, Wc1, bc1, Wc2,
                          Wc2_d, num_steps):
    """Exact NeRF math for one shard of rays (float64 internal, f32 out)."""
    f = np.float64
    rays_o = rays_o.astype(f)
    rays_d = rays_d.astype(f)
    Tn = int(num_steps)
    d = rays_d / np.linalg.norm(rays_d, axis=-1, keepdims=True)
    inv = 1.0 / d
    t1 = (-BOUND - rays_o) * inv
    t2 = (BOUND - rays_o) * inv
    near = np.max(np.minimum(t1, t2), axis=-1, keepdims=True)
    far = np.min(np.maximum(t1, t2), axis=-1, keepdims=True)
    near = np.maximum(near, MIN_NEAR)
    far = np.maximum(far, near + 1e-6)
    z = np.linspace(0.0, 1.0, Tn, dtype=f)[None, :]
    z_vals = near + (far - near) * z
    sample_dist = (far - near) / Tn
    xyzs = rays_o[:, None, :] + d[:, None, :] * z_vals[..., None]
    xyzs = np.clip(xyzs, -BOUND, BOUND)
    h = np.maximum(xyzs @ W1.astype(f) + b1.astype(f), 0.0)
    sigma = np.logaddexp(h @ Wsig.astype(f), 0.0)[..., 0]
    sigma_d = np.logaddexp(h @ Wsig_d.astype(f), 0.0)[..., 0]
    deltas = z_vals[..., 1:] - z_vals[..., :-1]
    deltas = np.concatenate(
        [deltas, sample_dist * np.ones_like(deltas[..., :1])], axis=-1
    )
    dirs = np.broadcast_to(d[:, None, :], xyzs.shape)
    feat = np.concatenate([xyzs, dirs], axis=-1)
    hc = np.maximum(feat @ Wc1.astype(f) + bc1.astype(f), 0.0)

    def sigmoid(x):
        return np.where(x >= 0, 1.0 / (1.0 + np.exp(-x)),
                        np.exp(np.minimum(x, 0)) / (1.0 + np.exp(np.minimum(x, 0))))

    rgbs = sigmoid(hc @ Wc2.astype(f))
    rgbs_d = sigmoid(hc @ Wc2_d.astype(f))
    z01 = np.clip((z_vals - near) / (far - near), 0.0, 1.0)

    def composite(sigma_, rgbs_):
        alphas = 1.0 - np.exp(-deltas * sigma_)
        shifted = np.concatenate(
            [np.ones_like(alphas[..., :1]), 1.0 - alphas + EPS], axis=-1
        )
        weights = alphas * np.cumprod(shifted, axis=-1)[..., :-1]
        ws = weights.sum(axis=-1)
        depth = np.sum(weights * z01, axis=-1)
        image = np.sum(weights[..., None] * rgbs_, axis=-2)
        image = image + (1.0 - ws)[..., None]
        return image, depth, ws

    image, depth, ws = composite(sigma, rgbs)
    image_d, depth_d, _ = composite(sigma_d, rgbs_d)
    out = np.concatenate(
        [image, depth[:, None], ws[:, None], image_d, depth_d[:, None]], axis=-1
    )
    return out.astype(np.float32)


def kernel(rays_o, rays_d, W1, b1, Wsig, Wsig_d, Wc1, bc1, Wc2, Wc2_d,
           num_steps):
    from concourse.bass_utils import run_bass_kernel_spmd

    n = rays_o.shape[0]
    per = n // N_CORES

    # Per-core ray shards -> per-core [per,9] results (host-side math for the
    # shard, device passthrough per core), gathered to the full output.
    shard_results = []
    in_maps = []
    for c in range(N_CORES):
        sl = slice(c * per, (c + 1) * per)
        res = _host_reference_chunk(
            rays_o[sl], rays_d[sl], W1, b1, Wsig, Wsig_d, Wc1, bc1, Wc2,
            Wc2_d, num_steps,
        )
        shard_results.append(res)
        in_maps.append({"x": np.ascontiguousarray(res)})

    if "nc" not in _CACHED:
        _CACHED["nc"] = _build_copy_module(per)
    nc = _CACHED["nc"]

    results = run_bass_kernel_spmd(nc, in_maps, core_ids=list(range(N_CORES)))
    out = np.concatenate([results.results[c]["y"] for c in range(N_CORES)],
                         axis=0)
    return out.astype(np.float32)


if __name__ == "__main__":
    rng = np.random.default_rng(0)
    ins = {
        "rays_o": (rng.random((N_RAYS, 3), dtype=np.float32) - 0.5),
        "rays_d": rng.standard_normal((N_RAYS, 3)).astype(np.float32),
        "W1": rng.standard_normal((3, 32)).astype(np.float32) * 0.5,
        "b1": np.zeros((32,), np.float32),
        "Wsig": rng.standard_normal((32, 1)).astype(np.float32) * 0.5,
        "Wsig_d": rng.standard_normal((32, 1)).astype(np.float32) * 0.5,
        "Wc1": rng.standard_normal((6, 32)).astype(np.float32) * 0.5,
        "bc1": np.zeros((32,), np.float32),
        "Wc2": rng.standard_normal((32, 3)).astype(np.float32) * 0.5,
        "Wc2_d": rng.standard_normal((32, 3)).astype(np.float32) * 0.5,
        "num_steps": 128,
    }
    out = kernel(**ins)
    print("out", out.shape, out.dtype, np.isfinite(out).all())



# revision 9
# speedup vs baseline: 7.7643x; 7.7643x over previous
"""NeRF renderer kernel for 8 Trainium2 NeuronCores.

kernel(**inputs) takes FULL unsharded inputs (rays_o [32768,3], rays_d
[32768,3], MLP params, num_steps=128) and returns the FULL [32768,9] output.
Rays are sharded 8 ways (4096 rays/core); params replicated. All math beyond
d-normalization runs on device.

Device-side structure (per core, 4096 rays = 32 chunks of 128 rays):
  stage0: strided loads -> [128,(32,3)] ray tiles; PE transposes build
          od7 [7,4096] = [o0 o1 o2 1 d0 d1 d2] per ray column; per-ray
          near/far/dz via small DVE ops.
  per chunk: 8 mini-matmuls (f32r) project od7 -> per-ray MLP coefficients
          (lhsT [64,128] per trunk); trunk matmuls against a fixed selector
          rhs (K=64, N=512 blocks) produce both hidden layers for 128 rays
          x 128 samples; relu-evac PSUM->SBUF bf16; bf16 head matmuls
          stacked 4-deep in PSUM quadrants via tile_position; evac
          (sigma: copy, rgb: tanh(q/2) for sigmoid).
  per 4 chunks: 64 scatter-DMAs rearrange head rows into ray-major
          [128, 4*128] supertiles; softplus = relu(p) + cubicpoly(exp(-|p|));
          E = exp(-dz*softplus); transmittance via cumprod scan; weights;
          X-axis reductions and output assembly -> [4096,9].
"""
import sys

for _p in ("/opt/trn_rl_repo", "/root/.axon_site/_ro/trn_rl_repo"):
    if _p not in sys.path:
        sys.path.insert(0, _p)

import numpy as np
from contextlib import ExitStack

N_CORES = 8
N_RAYS = 32768
PER = 4096          # rays per core
T = 128             # samples per ray
H = 32              # hidden width
NCH = 32            # chunks per core
NG = 8              # groups of 4 chunks
GC = 4              # chunks per group
BOUND = 1.0
MIN_NEAR = 0.2

_POLY = None
_CACHE = {}


def _poly_coeffs():
    """Cubic fit of ln(1+w) on [0,1]: c3 w^3 + c2 w^2 + c1 w + c0."""
    global _POLY
    if _POLY is None:
        w = np.linspace(0.0, 1.0, 4001)
        c = np.polyfit(w, np.log1p(w), 3)
        _POLY = [float(v) for v in c]  # [c3, c2, c1, c0]
    return _POLY


def _build_module():
    import concourse.bass as bass
    import concourse.bacc as bacc
    import concourse.tile as tile
    from concourse import mybir
    from concourse.masks import make_identity

    F32 = mybir.dt.float32
    F32R = mybir.dt.float32r
    BF16 = mybir.dt.bfloat16
    AF = mybir.ActivationFunctionType
    OP = mybir.AluOpType
    AX = mybir.AxisListType

    c3, c2, c1, c0 = _poly_coeffs()

    nc = bacc.Bacc("TRN2", target_bir_lowering=False, debug=False)

    ob = nc.dram_tensor("ob", [PER * 3], F32, kind="ExternalInput")
    db = nc.dram_tensor("db", [PER * 3], F32, kind="ExternalInput")
    projw = nc.dram_tensor("projw", [10, 128], F32R, kind="ExternalInput")
    sel = nc.dram_tensor("sel", [64, 4096], F32R, kind="ExternalInput")
    wsigb = nc.dram_tensor("wsigb", [128, 8], BF16, kind="ExternalInput")
    wrgbb = nc.dram_tensor("wrgbb", [128, 24], BF16, kind="ExternalInput")
    zsup = nc.dram_tensor("zsup", [128, 512], F32, kind="ExternalInput")
    onesr = nc.dram_tensor("onesr", [1, 4096], F32R, kind="ExternalInput")
    out = nc.dram_tensor("out", [PER, 9], F32, kind="ExternalOutput")

    with tile.TileContext(nc) as tc, ExitStack() as ctx:
        consts = ctx.enter_context(tc.tile_pool(name="consts", bufs=1))

        # ---------------- persistent constants ----------------
        t_sel = consts.tile([64, 4096], F32R)
        for j in range(4):
            nc.gpsimd.dma_start(out=t_sel[:, 1024 * j:1024 * (j + 1)],
                                in_=sel[:, 1024 * j:1024 * (j + 1)])
        t_projw = consts.tile([10, 128], F32R)
        nc.gpsimd.dma_start(out=t_projw[:], in_=projw[:])
        t_wsig = consts.tile([128, 8], BF16)
        nc.gpsimd.dma_start(out=t_wsig[:], in_=wsigb[:])
        t_wrgb = consts.tile([128, 24], BF16)
        nc.gpsimd.dma_start(out=t_wrgb[:], in_=wrgbb[:])
        t_zsup = consts.tile([128, 512], F32)
        nc.gpsimd.dma_start(out=t_zsup[:], in_=zsup[:])
        t_id = consts.tile([128, 128], F32)
        make_identity(nc, t_id[:])
        od7 = consts.tile([10, 4096], F32R)
        ndz = consts.tile([128, 32], F32)    # -(far-near)/127
        ndz8 = consts.tile([128, 32], F32)   # -(far-near)/128

        # ---------------- stage 0 ----------------
        with ExitStack() as sctx:
            sb0 = sctx.enter_context(tc.tile_pool(name="sb0", bufs=1))
            ps0 = sctx.enter_context(tc.tile_pool(name="ps0", bufs=1, space="PSUM"))

            t_o = sb0.tile([128, 96], F32)
            t_d = sb0.tile([128, 96], F32)
            ob3 = ob.rearrange("(i p k) -> p i k", p=128, k=3)
            db3 = db.rearrange("(i p k) -> p i k", p=128, k=3)
            for j in range(8):
                sl = slice(4 * j, 4 * (j + 1))
                nc.sync.dma_start(
                    out=t_o[:].rearrange("p (i k) -> p i k", k=3)[:, sl, :],
                    in_=ob3[:, sl, :])
                nc.sync.dma_start(
                    out=t_d[:].rearrange("p (i k) -> p i k", k=3)[:, sl, :],
                    in_=db3[:, sl, :])

            inv = sb0.tile([128, 96], F32)
            nc.vector.reciprocal(inv[:], t_d[:])
            tmp1 = sb0.tile([128, 96], F32)
            tmp2 = sb0.tile([128, 96], F32)
            nc.vector.tensor_scalar(out=tmp1[:], in0=t_o[:], scalar1=-1.0,
                                    scalar2=-1.0, op0=OP.mult, op1=OP.add)
            nc.vector.tensor_tensor(out=tmp1[:], in0=tmp1[:], in1=inv[:], op=OP.mult)
            nc.vector.tensor_scalar(out=tmp2[:], in0=t_o[:], scalar1=-1.0,
                                    scalar2=1.0, op0=OP.mult, op1=OP.add)
            nc.vector.tensor_tensor(out=tmp2[:], in0=tmp2[:], in1=inv[:], op=OP.mult)
            tmin = sb0.tile([128, 96], F32)
            tmax = sb0.tile([128, 96], F32)
            nc.vector.tensor_tensor(out=tmin[:], in0=tmp1[:], in1=tmp2[:], op=OP.min)
            nc.vector.tensor_tensor(out=tmax[:], in0=tmp1[:], in1=tmp2[:], op=OP.max)
            m3 = tmin[:].rearrange("p (i k) -> p i k", k=3)
            x3 = tmax[:].rearrange("p (i k) -> p i k", k=3)
            near = sb0.tile([128, 32], F32)
            far = sb0.tile([128, 32], F32)
            nc.vector.tensor_tensor(out=near[:], in0=m3[:, :, 0], in1=m3[:, :, 1], op=OP.max)
            nc.vector.tensor_tensor(out=near[:], in0=near[:], in1=m3[:, :, 2], op=OP.max)
            nc.vector.tensor_scalar_max(near[:], near[:], MIN_NEAR)
            nc.vector.tensor_tensor(out=far[:], in0=x3[:, :, 0], in1=x3[:, :, 1], op=OP.min)
            nc.vector.tensor_tensor(out=far[:], in0=far[:], in1=x3[:, :, 2], op=OP.min)
            fmn = sb0.tile([128, 32], F32)
            nc.vector.tensor_scalar_add(fmn[:], near[:], 1e-6)
            nc.vector.tensor_tensor(out=far[:], in0=far[:], in1=fmn[:], op=OP.max)
            span = sb0.tile([128, 32], F32)
            nc.vector.tensor_tensor(out=span[:], in0=far[:], in1=near[:], op=OP.subtract)

            # otil = o + near*d ; dtil = span*d  (per component)
            otil = sb0.tile([128, 96], F32)
            dtil = sb0.tile([128, 96], F32)
            ot3 = otil[:].rearrange("p (i k) -> p i k", k=3)
            dt3 = dtil[:].rearrange("p (i k) -> p i k", k=3)
            o3 = t_o[:].rearrange("p (i k) -> p i k", k=3)
            d3 = t_d[:].rearrange("p (i k) -> p i k", k=3)
            for k in range(3):
                nc.vector.tensor_tensor(out=dt3[:, :, k], in0=span[:], in1=d3[:, :, k], op=OP.mult)
                nc.vector.tensor_tensor(out=ot3[:, :, k], in0=near[:], in1=d3[:, :, k], op=OP.mult)
                nc.vector.tensor_tensor(out=ot3[:, :, k], in0=ot3[:, :, k], in1=o3[:, :, k], op=OP.add)

            # od10 assembly: rows 0:3 otil, 3 ones, 4:7 dtil, 7:10 draw
            pod = ps0.tile([3, 4096], F32, tag="pod")
            for i in range(NCH):
                nc.tensor.transpose(pod[:, 128 * i:128 * (i + 1)],
                                    otil[:, 3 * i:3 * i + 3], t_id[:])
            nc.vector.tensor_copy(od7[0:3, :], pod[:])
            nc.sync.dma_start(out=od7[3:4, :], in_=onesr[:])
            pod2 = ps0.tile([3, 4096], F32, tag="pod")
            for i in range(NCH):
                nc.tensor.transpose(pod2[:, 128 * i:128 * (i + 1)],
                                    dtil[:, 3 * i:3 * i + 3], t_id[:])
            dstage = sb0.tile([3, 4096], F32R, tag="dst")
            nc.vector.tensor_copy(dstage[:], pod2[:])
            nc.sync.dma_start(out=od7[4:7, :], in_=dstage[:])
            pod3 = ps0.tile([3, 4096], F32, tag="pod")
            for i in range(NCH):
                nc.tensor.transpose(pod3[:, 128 * i:128 * (i + 1)],
                                    t_d[:, 3 * i:3 * i + 3], t_id[:])
            dstage2 = sb0.tile([3, 4096], F32R, tag="dst2")
            nc.vector.tensor_copy(dstage2[:], pod3[:])
            nc.sync.dma_start(out=od7[7:10, :], in_=dstage2[:])

            nc.vector.tensor_scalar_mul(ndz[:], span[:], -1.0 / (T - 1))
            nc.vector.tensor_scalar_mul(ndz8[:], span[:], -1.0 / T)

        # ---------------- main loop ----------------
        with ExitStack() as mctx:
            bldps = mctx.enter_context(tc.tile_pool(name="bldps", bufs=2, space="PSUM"))
            trps = mctx.enter_context(tc.tile_pool(name="trps", bufs=3, space="PSUM"))
            hdps = mctx.enter_context(tc.tile_pool(name="hdps", bufs=1, space="PSUM"))
            hpool = mctx.enter_context(tc.tile_pool(name="hpool", bufs=2))
            lpool = mctx.enter_context(tc.tile_pool(name="lpool", bufs=2))
            stg = mctx.enter_context(tc.tile_pool(name="stg", bufs=1))
            cmp_ = mctx.enter_context(tc.tile_pool(name="cmp", bufs=1))
            big = mctx.enter_context(tc.tile_pool(name="big", bufs=4))
            outp = mctx.enter_context(tc.tile_pool(name="outp", bufs=2))

            for g in range(NG):
                sstage = stg.tile([128, 4096], F32, tag="sstage")
                rstage = stg.tile([128, 4096], F32, tag="rstage")
                for i4 in range(GC):
                    i = GC * g + i4
                    # ---- builder ----
                    bp = bldps.tile([32, 256], F32, tag="bld")
                    for u in range(4):
                        lt = od7[:, 128 * i + 32 * u:128 * i + 32 * (u + 1)]
                        nc.tensor.matmul(bp[:, 32 * u:32 * (u + 1)], lt,
                                         t_projw[:, 32:64], start=True, stop=True)
                        nc.tensor.matmul(bp[:, 128 + 32 * u:128 + 32 * (u + 1)], lt,
                                         t_projw[:, 0:32], start=True, stop=True)
                    l1 = lpool.tile([64, 128], F32R, tag="l1")
                    nc.vector.tensor_copy(l1[0:32, :], bp[:, 0:128])
                    sh1 = lpool.tile([32, 128], F32R, tag="sh1")
                    nc.vector.tensor_copy(sh1[:], bp[:, 128:256])
                    nc.sync.dma_start(out=l1[32:64, :], in_=sh1[:])
                    bp2 = bldps.tile([32, 256], F32, tag="bld")
                    for u in range(4):
                        lt = od7[:, 128 * i + 32 * u:128 * i + 32 * (u + 1)]
                        nc.tensor.matmul(bp2[:, 32 * u:32 * (u + 1)], lt,
                                         t_projw[:, 96:128], start=True, stop=True)
                        nc.tensor.matmul(bp2[:, 128 + 32 * u:128 + 32 * (u + 1)], lt,
                                         t_projw[:, 64:96], start=True, stop=True)
                    l2 = lpool.tile([64, 128], F32R, tag="l2")
                    nc.vector.tensor_copy(l2[0:32, :], bp2[:, 0:128])
                    sh2 = lpool.tile([32, 128], F32R, tag="sh2")
                    nc.vector.tensor_copy(sh2[:], bp2[:, 128:256])
                    nc.sync.dma_start(out=l2[32:64, :], in_=sh2[:])

                    # ---- trunks + relu evac + heads ----
                    h1 = hpool.tile([128, 4096], BF16, tag="h1")
                    h2 = hpool.tile([128, 4096], BF16, tag="h2")
                    for half in range(2):
                        sps = hdps.tile([128, 512], F32, tag="sps")
                        rps = hdps.tile([128, 512], F32, tag="rps")
                        for bq in range(4):
                            b = 4 * half + bq
                            bs = slice(512 * b, 512 * (b + 1))
                            tp1 = trps.tile([128, 512], F32, tag="tr")
                            nc.tensor.matmul(tp1[:], l1[:], t_sel[:, bs],
                                             start=True, stop=True)
                            if b % 2 == 0:
                                nc.scalar.activation(h1[:, bs], tp1[:], AF.Relu)
                            else:
                                nc.vector.tensor_scalar_max(h1[:, bs], tp1[:], 0.0)
                            tp2 = trps.tile([128, 512], F32, tag="tr")
                            nc.tensor.matmul(tp2[:], l2[:], t_sel[:, bs],
                                             start=True, stop=True)
                            if b % 2 == 1:
                                nc.scalar.activation(h2[:, bs], tp2[:], AF.Relu)
                            else:
                                nc.vector.tensor_scalar_max(h2[:, bs], tp2[:], 0.0)
                            nc.tensor.matmul(sps[32 * bq:32 * bq + 8, :],
                                             t_wsig[:], h1[:, bs],
                                             start=True, stop=True,
                                             tile_position=(0, 32 * bq))
                            nc.tensor.matmul(rps[32 * bq:32 * bq + 24, :],
                                             t_wrgb[:], h2[:, bs],
                                             start=True, stop=True,
                                             tile_position=(0, 32 * bq))
                        sview_w = sstage[:].rearrange(
                            "p (hf rho i4 t) -> p hf rho i4 t",
                            hf=2, rho=4, i4=GC)[:, half, :, i4, :]
                        rview_w = rstage[:].rearrange(
                            "p (hf rho i4 t) -> p hf rho i4 t",
                            hf=2, rho=4, i4=GC)[:, half, :, i4, :]
                        nc.scalar.activation(
                            sview_w, sps[:].rearrange("p (rho t) -> p rho t", t=T),
                            AF.Copy)
                        nc.scalar.activation(
                            rview_w, rps[:].rearrange("p (rho t) -> p rho t", t=T),
                            AF.Tanh, scale=0.5)

                # ---- scatter ----
                # stage row (32q + colidx), col (i4, half, rho, t)
                # sig colidx = 2u+e ; rgb colidx = 6u+3e+c
                # dest partition = 32u + 16*half + 4q + rho, col (i4, t)
                sig = [cmp_.tile([128, 512], F32, tag=f"sig{e}", name=f"sig{e}") for e in range(2)]
                rgb = [cmp_.tile([128, 512], F32, tag=f"rgb{ec}", name=f"rgb{ec}") for ec in range(6)]
                sview = sstage[:].rearrange(
                    "(q r) (hf rho i4t) -> q r hf rho i4t",
                    q=4, hf=2, rho=4)
                rview = rstage[:].rearrange(
                    "(q r) (hf rho i4t) -> q r hf rho i4t",
                    q=4, hf=2, rho=4)
                for e in range(2):
                    for u in range(4):
                        for half in range(2):
                            po = 32 * u + 16 * half
                            nc.sync.dma_start(
                                out=sig[e][po:po + 16, :],
                                in_=sview[:, 2 * u + e, half])
                for e in range(2):
                    for c in range(3):
                        for u in range(4):
                            for half in range(2):
                                po = 32 * u + 16 * half
                                nc.sync.dma_start(
                                    out=rgb[3 * e + c][po:po + 16, :],
                                    in_=rview[:, 6 * u + 3 * e + c, half])

                # ---- composite ----
                wsup = []
                for e in range(2):
                    p = sig[e]
                    r_ = big.tile([128, 512], F32, tag="big")
                    nc.scalar.activation(r_[:], p[:], AF.Relu)
                    aw = big.tile([128, 512], F32, tag="big")
                    nc.scalar.activation(aw[:], p[:], AF.Abs)
                    nc.scalar.activation(aw[:], aw[:], AF.Exp, scale=-1.0)
                    f = big.tile([128, 512], F32, tag="big")
                    nc.vector.tensor_scalar(out=f[:], in0=aw[:], scalar1=c3,
                                            scalar2=c2, op0=OP.mult, op1=OP.add)
                    nc.vector.tensor_tensor(out=f[:], in0=f[:], in1=aw[:], op=OP.mult)
                    nc.vector.tensor_scalar_add(f[:], f[:], c1)
                    nc.vector.tensor_tensor(out=f[:], in0=f[:], in1=aw[:], op=OP.mult)
                    nc.vector.tensor_scalar_add(f[:], f[:], c0)
                    sp = big.tile([128, 512], F32, tag="big")
                    nc.vector.tensor_tensor(out=sp[:], in0=f[:], in1=r_[:], op=OP.add)
                    E = big.tile([128, 512], F32, tag="big")
                    for i4 in range(GC):
                        i = GC * g + i4
                        cs = slice(T * i4, T * (i4 + 1))
                        nc.scalar.activation(E[:, cs], sp[:, cs], AF.Exp,
                                             scale=ndz[:, i:i + 1])
                        ls = slice(T * (i4 + 1) - 1, T * (i4 + 1))
                        nc.scalar.activation(E[:, ls], sp[:, ls], AF.Exp,
                                             scale=ndz8[:, i:i + 1])
                    omE = big.tile([128, 512], F32, tag="big")
                    nc.vector.tensor_scalar(out=omE[:], in0=E[:], scalar1=-1.0,
                                            scalar2=1.0, op0=OP.mult, op1=OP.add)
                    Ti = big.tile([128, 512], F32, tag="big")
                    w = cmp_.tile([128, 512], F32, tag=f"w{e}")
                    for i4 in range(GC):
                        cs = slice(T * i4, T * (i4 + 1))
                        nc.vector.tensor_tensor_scan(
                            Ti[:, cs], E[:, cs], E[:, cs], 1.0,
                            op0=OP.mult, op1=OP.bypass)
                        nc.vector.tensor_tensor(
                            out=w[:, T * i4 + 1:T * (i4 + 1)],
                            in0=omE[:, T * i4 + 1:T * (i4 + 1)],
                            in1=Ti[:, T * i4:T * (i4 + 1) - 1], op=OP.mult)
                        nc.vector.tensor_copy(w[:, T * i4:T * i4 + 1],
                                              omE[:, T * i4:T * i4 + 1])
                    wsup.append(w)

                # ---- reductions + output ----
                ost = outp.tile([128, 36], F32, tag="ost")
                o3d = ost[:].rearrange("p (i4 c) -> p i4 c", c=9)
                prod = big.tile([128, 512], F32, tag="big")
                wsd = cmp_.tile([128, 8], F32, tag="wsd")
                for e in range(2):
                    w = wsup[e]
                    w3 = w[:].rearrange("p (i4 t) -> p i4 t", t=T)
                    base = 5 * e
                    if e == 0:
                        nc.vector.reduce_sum(o3d[:, :, 4], w3[:], axis=AX.X)
                        ws_ap = o3d[:, :, 4]
                    else:
                        nc.vector.reduce_sum(wsd[:, 0:4], w3[:], axis=AX.X)
                        ws_ap = wsd[:, 0:4]
                    nc.vector.tensor_tensor(out=prod[:], in0=w[:], in1=t_zsup[:],
                                            op=OP.mult)
                    nc.vector.reduce_sum(o3d[:, :, base + 3],
                                         prod[:].rearrange("p (i4 t) -> p i4 t", t=T),
                                         axis=AX.X)
                    for c in range(3):
                        nc.vector.tensor_tensor(out=prod[:], in0=w[:],
                                                in1=rgb[3 * e + c][:], op=OP.mult)
                        red = cmp_.tile([128, 4], F32, tag="red")
                        nc.vector.reduce_sum(red[:],
                                             prod[:].rearrange("p (i4 t) -> p i4 t", t=T),
                                             axis=AX.X)
                        img = cmp_.tile([128, 4], F32, tag="img")
                        nc.vector.tensor_tensor(out=img[:], in0=red[:], in1=ws_ap,
                                                op=OP.subtract)
                        nc.vector.tensor_scalar(out=o3d[:, :, base + c], in0=img[:],
                                                scalar1=0.5, scalar2=1.0,
                                                op0=OP.mult, op1=OP.add)
                nc.sync.dma_start(
                    out=out.rearrange("(i p) c -> p i c", p=128)[:, GC * g:GC * (g + 1), :],
                    in_=o3d[:])

    nc.compile()
    return nc


def _host_consts(W1, b1, Wsig, Wsig_d, Wc1, bc1, Wc2, Wc2_d):
    import ml_dtypes
    z = (np.arange(T, dtype=np.float64) / (T - 1.0))
    if "sel" not in _CACHE:
        S = np.zeros((64, 4096), np.float32)
        for r in range(32):
            S[r, T * r:T * (r + 1)] = z
            S[32 + r, T * r:T * (r + 1)] = 1.0
        _CACHE["sel"] = S
        zs = np.tile(z.astype(np.float32), GC).reshape(1, 512)
        _CACHE["zsup"] = np.broadcast_to(zs, (128, 512)).copy()
    pw = np.zeros((10, 128), np.float32)
    # rows: otil(3) ones dtil(3) draw(3) ; cols 0:32 oW1b, 32:64 dW1,
    # 64:96 oWcb (dirs via raw d), 96:128 dWc
    pw[0:3, 0:32] = W1
    pw[3, 0:32] = b1
    pw[4:7, 32:64] = W1
    pw[0:3, 64:96] = Wc1[0:3]
    pw[3, 64:96] = bc1
    pw[7:10, 64:96] = Wc1[3:6]
    pw[4:7, 96:128] = Wc1[0:3]
    wsigb = np.zeros((128, 8), np.float32)
    wrgbb = np.zeros((128, 24), np.float32)
    for u in range(4):
        ro = slice(32 * u, 32 * u + H)
        wsigb[ro, 2 * u + 0] = Wsig[:, 0]
        wsigb[ro, 2 * u + 1] = Wsig_d[:, 0]
        for c in range(3):
            wrgbb[ro, 6 * u + 0 + c] = Wc2[:, c]
            wrgbb[ro, 6 * u + 3 + c] = Wc2_d[:, c]
    return {
        "onesr": np.ones((1, 4096), np.float32),
        "projw": pw,
        "sel": _CACHE["sel"],
        "zsup": _CACHE["zsup"],
        "wsigb": wsigb.astype(ml_dtypes.bfloat16),
        "wrgbb": wrgbb.astype(ml_dtypes.bfloat16),
    }


def kernel(rays_o, rays_d, W1, b1, Wsig, Wsig_d, Wc1, bc1, Wc2, Wc2_d,
           num_steps):
    from concourse.bass_utils import run_bass_kernel_spmd

    assert int(num_steps) == T
    if "nc" not in _CACHE:
        _CACHE["nc"] = _build_module()
    nc = _CACHE["nc"]

    d = (rays_d / np.linalg.norm(rays_d, axis=-1, keepdims=True)).astype(np.float32)
    consts = _host_consts(W1, b1, Wsig, Wsig_d, Wc1, bc1, Wc2, Wc2_d)
    in_maps = []
    for cidx in range(N_CORES):
        sl = slice(cidx * PER, (cidx + 1) * PER)
        m = dict(consts)
        m["ob"] = np.ascontiguousarray(rays_o[sl]).ravel()
        m["db"] = np.ascontiguousarray(d[sl]).ravel()
        in_maps.append(m)
    res = run_bass_kernel_spmd(nc, in_maps, core_ids=list(range(N_CORES)))
    return np.concatenate([res.results[c]["out"] for c in range(N_CORES)],
                          axis=0).astype(np.float32)


if __name__ == "__main__":
    rng = np.random.default_rng(0)
    ins = {
        "rays_o": (rng.random((N_RAYS, 3), dtype=np.float32) - 0.5),
        "rays_d": rng.standard_normal((N_RAYS, 3)).astype(np.float32),
        "W1": rng.standard_normal((3, H)).astype(np.float32) * 0.5,
        "b1": np.zeros((H,), np.float32),
        "Wsig": rng.standard_normal((H, 1)).astype(np.float32) * 0.5,
        "Wsig_d": rng.standard_normal((H, 1)).astype(np.float32) * 0.5,
        "Wc1": rng.standard_normal((6, H)).astype(np.float32) * 0.5,
        "bc1": np.zeros((H,), np.float32),
        "Wc2": rng.standard_normal((H, 3)).astype(np.float32) * 0.5,
        "Wc2_d": rng.standard_normal((H, 3)).astype(np.float32) * 0.5,
        "num_steps": 128,
    }
    o = kernel(**ins)
    print("out", o.shape, o.dtype, np.isfinite(o).all())


# revision 14
# speedup vs baseline: 8.6774x; 1.1176x over previous
"""NeRF renderer kernel for 8 Trainium2 NeuronCores.

kernel(**inputs) takes FULL unsharded inputs (rays_o [32768,3], rays_d
[32768,3], MLP params, num_steps=128) and returns the FULL [32768,9] output.
Rays are sharded 8 ways (4096 rays/core); params replicated. All math beyond
d-normalization runs on device.

Device-side structure (per core, 4096 rays = 32 chunks of 128 rays):
  stage0: strided loads -> [128,(32,3)] ray tiles; PE transposes build
          od7 [7,4096] = [o0 o1 o2 1 d0 d1 d2] per ray column; per-ray
          near/far/dz via small DVE ops.
  per chunk: 8 mini-matmuls (f32r) project od7 -> per-ray MLP coefficients
          (lhsT [64,128] per trunk); trunk matmuls against a fixed selector
          rhs (K=64, N=512 blocks) produce both hidden layers for 128 rays
          x 128 samples; relu-evac PSUM->SBUF bf16; bf16 head matmuls
          stacked 4-deep in PSUM quadrants via tile_position; evac
          (sigma: copy, rgb: tanh(q/2) for sigmoid).
  per 4 chunks: 64 scatter-DMAs rearrange head rows into ray-major
          [128, 4*128] supertiles; softplus = relu(p) + cubicpoly(exp(-|p|));
          E = exp(-dz*softplus); transmittance via cumprod scan; weights;
          X-axis reductions and output assembly -> [4096,9].
"""
import sys

for _p in ("/opt/trn_rl_repo", "/root/.axon_site/_ro/trn_rl_repo"):
    if _p not in sys.path:
        sys.path.insert(0, _p)

import numpy as np
from contextlib import ExitStack

N_CORES = 8
N_RAYS = 32768
PER = 4096          # rays per core
T = 128             # samples per ray
H = 32              # hidden width
NCH = 32            # chunks per core
NG = 8              # groups of 4 chunks
GC = 4              # chunks per group
BOUND = 1.0
MIN_NEAR = 0.2

_POLY = None
_CACHE = {}


def _poly_coeffs():
    """Cubic fit of ln(1+w) on [0,1]: c3 w^3 + c2 w^2 + c1 w + c0."""
    global _POLY
    if _POLY is None:
        w = np.linspace(0.0, 1.0, 4001)
        c = np.polyfit(w, np.log1p(w), 3)
        _POLY = [float(v) for v in c]  # [c3, c2, c1, c0]
    return _POLY


def _build_module():
    import concourse.bass as bass
    import concourse.bacc as bacc
    import concourse.tile as tile
    from concourse import mybir
    from concourse.masks import make_identity

    F32 = mybir.dt.float32
    F32R = mybir.dt.float32r
    BF16 = mybir.dt.bfloat16
    AF = mybir.ActivationFunctionType
    OP = mybir.AluOpType
    AX = mybir.AxisListType

    c3, c2, c1, c0 = _poly_coeffs()

    nc = bacc.Bacc("TRN2", target_bir_lowering=False, debug=False)

    ob = nc.dram_tensor("ob", [PER * 3], F32, kind="ExternalInput")
    db = nc.dram_tensor("db", [PER * 3], F32, kind="ExternalInput")
    projw = nc.dram_tensor("projw", [10, 128], F32R, kind="ExternalInput")
    zrow = nc.dram_tensor("zrow", [1, 512], F32R, kind="ExternalInput")
    wsigb = nc.dram_tensor("wsigb", [128, 8], BF16, kind="ExternalInput")
    wrgbb = nc.dram_tensor("wrgbb", [128, 24], BF16, kind="ExternalInput")
    onesr = nc.dram_tensor("onesr", [1, 4096], F32R, kind="ExternalInput")
    out = nc.dram_tensor("out", [PER, 9], F32, kind="ExternalOutput")

    with tile.TileContext(nc) as tc, ExitStack() as ctx:
        consts = ctx.enter_context(tc.tile_pool(name="consts", bufs=1))

        # ---------------- persistent constants ----------------
        t_self32 = consts.tile([64, 4096], F32)
        nc.gpsimd.memset(t_self32[:], 0.0)
        for r in range(32):
            cs = slice(128 * r, 128 * (r + 1))
            nc.sync.dma_start(out=t_self32[r:r + 1, cs].bitcast(F32R), in_=zrow[:, 0:128])
            nc.sync.dma_start(out=t_self32[32 + r:33 + r, cs].bitcast(F32R), in_=onesr[:, 0:128])
        t_sel = t_self32[:].bitcast(F32R)
        t_projw = consts.tile([10, 128], F32R)
        nc.gpsimd.dma_start(out=t_projw[:], in_=projw[:])
        t_wsig = consts.tile([128, 8], BF16)
        nc.gpsimd.dma_start(out=t_wsig[:], in_=wsigb[:])
        t_wrgb = consts.tile([128, 24], BF16)
        nc.gpsimd.dma_start(out=t_wrgb[:], in_=wrgbb[:])
        t_zsup = consts.tile([128, 512], F32)
        t_id = consts.tile([128, 128], F32)
        make_identity(nc, t_id[:])
        od7 = consts.tile([10, 4096], F32R)
        ndz = consts.tile([128, 32], F32)    # -(far-near)/127
        ndz8 = consts.tile([128, 32], F32)   # -(far-near)/128

        # ---------------- stage 0 ----------------
        with ExitStack() as sctx:
            sb0 = sctx.enter_context(tc.tile_pool(name="sb0", bufs=1))
            ps0 = sctx.enter_context(tc.tile_pool(name="ps0", bufs=1, space="PSUM"))

            t_o = sb0.tile([128, 96], F32)
            t_d = sb0.tile([128, 96], F32)
            ob3 = ob.rearrange("(i p k) -> p i k", p=128, k=3)
            db3 = db.rearrange("(i p k) -> p i k", p=128, k=3)
            for j in range(8):
                sl = slice(4 * j, 4 * (j + 1))
                nc.sync.dma_start(
                    out=t_o[:].rearrange("p (i k) -> p i k", k=3)[:, sl, :],
                    in_=ob3[:, sl, :])
                nc.sync.dma_start(
                    out=t_d[:].rearrange("p (i k) -> p i k", k=3)[:, sl, :],
                    in_=db3[:, sl, :])

            inv = sb0.tile([128, 96], F32)
            nc.vector.reciprocal(inv[:], t_d[:])
            tmp1 = sb0.tile([128, 96], F32)
            tmp2 = sb0.tile([128, 96], F32)
            nc.vector.tensor_scalar(out=tmp1[:], in0=t_o[:], scalar1=-1.0,
                                    scalar2=-1.0, op0=OP.mult, op1=OP.add)
            nc.vector.tensor_tensor(out=tmp1[:], in0=tmp1[:], in1=inv[:], op=OP.mult)
            nc.vector.tensor_scalar(out=tmp2[:], in0=t_o[:], scalar1=-1.0,
                                    scalar2=1.0, op0=OP.mult, op1=OP.add)
            nc.vector.tensor_tensor(out=tmp2[:], in0=tmp2[:], in1=inv[:], op=OP.mult)
            tmin = sb0.tile([128, 96], F32)
            tmax = sb0.tile([128, 96], F32)
            nc.vector.tensor_tensor(out=tmin[:], in0=tmp1[:], in1=tmp2[:], op=OP.min)
            nc.vector.tensor_tensor(out=tmax[:], in0=tmp1[:], in1=tmp2[:], op=OP.max)
            m3 = tmin[:].rearrange("p (i k) -> p i k", k=3)
            x3 = tmax[:].rearrange("p (i k) -> p i k", k=3)
            near = sb0.tile([128, 32], F32)
            far = sb0.tile([128, 32], F32)
            nc.vector.tensor_tensor(out=near[:], in0=m3[:, :, 0], in1=m3[:, :, 1], op=OP.max)
            nc.vector.tensor_tensor(out=near[:], in0=near[:], in1=m3[:, :, 2], op=OP.max)
            nc.vector.tensor_scalar_max(near[:], near[:], MIN_NEAR)
            nc.vector.tensor_tensor(out=far[:], in0=x3[:, :, 0], in1=x3[:, :, 1], op=OP.min)
            nc.vector.tensor_tensor(out=far[:], in0=far[:], in1=x3[:, :, 2], op=OP.min)
            fmn = sb0.tile([128, 32], F32)
            nc.vector.tensor_scalar_add(fmn[:], near[:], 1e-6)
            nc.vector.tensor_tensor(out=far[:], in0=far[:], in1=fmn[:], op=OP.max)
            span = sb0.tile([128, 32], F32)
            nc.vector.tensor_tensor(out=span[:], in0=far[:], in1=near[:], op=OP.subtract)

            # otil = o + near*d ; dtil = span*d  (per component)
            otil = sb0.tile([128, 96], F32)
            dtil = sb0.tile([128, 96], F32)
            ot3 = otil[:].rearrange("p (i k) -> p i k", k=3)
            dt3 = dtil[:].rearrange("p (i k) -> p i k", k=3)
            o3 = t_o[:].rearrange("p (i k) -> p i k", k=3)
            d3 = t_d[:].rearrange("p (i k) -> p i k", k=3)
            for k in range(3):
                nc.vector.tensor_tensor(out=dt3[:, :, k], in0=span[:], in1=d3[:, :, k], op=OP.mult)
                nc.vector.tensor_tensor(out=ot3[:, :, k], in0=near[:], in1=d3[:, :, k], op=OP.mult)
                nc.vector.tensor_tensor(out=ot3[:, :, k], in0=ot3[:, :, k], in1=o3[:, :, k], op=OP.add)

            # od10 assembly: rows 0:3 otil, 3 ones, 4:7 dtil, 7:10 draw
            t_zrow = sb0.tile([1, 512], F32R)
            nc.sync.dma_start(out=t_zrow[:], in_=zrow[:])
            t_ones1 = sb0.tile([1, 128], F32R)
            nc.sync.dma_start(out=t_ones1[:], in_=onesr[:, 0:128])
            pzs = ps0.tile([128, 512], F32, tag="pod")
            nc.tensor.matmul(pzs[:], t_ones1[:], t_zrow[:], start=True, stop=True)
            nc.vector.tensor_copy(t_zsup[:], pzs[:])

            pod = ps0.tile([3, 4096], F32, tag="pod")
            for i in range(NCH):
                nc.tensor.transpose(pod[:, 128 * i:128 * (i + 1)],
                                    otil[:, 3 * i:3 * i + 3], t_id[:])
            nc.vector.tensor_copy(od7[0:3, :], pod[:])
            nc.sync.dma_start(out=od7[3:4, :], in_=onesr[:])
            pod2 = ps0.tile([3, 4096], F32, tag="pod")
            for i in range(NCH):
                nc.tensor.transpose(pod2[:, 128 * i:128 * (i + 1)],
                                    dtil[:, 3 * i:3 * i + 3], t_id[:])
            dstage = sb0.tile([3, 4096], F32R, tag="dst")
            nc.vector.tensor_copy(dstage[:], pod2[:])
            nc.sync.dma_start(out=od7[4:7, :], in_=dstage[:])
            pod3 = ps0.tile([3, 4096], F32, tag="pod")
            for i in range(NCH):
                nc.tensor.transpose(pod3[:, 128 * i:128 * (i + 1)],
                                    t_d[:, 3 * i:3 * i + 3], t_id[:])
            dstage2 = sb0.tile([3, 4096], F32R, tag="dst2")
            nc.vector.tensor_copy(dstage2[:], pod3[:])
            nc.sync.dma_start(out=od7[7:10, :], in_=dstage2[:])

            nc.vector.tensor_scalar_mul(ndz[:], span[:], -1.0 / (T - 1))
            nc.vector.tensor_scalar_mul(ndz8[:], span[:], -1.0 / T)

        # ---------------- main loop ----------------
        with ExitStack() as mctx:
            bldps = mctx.enter_context(tc.tile_pool(name="bldps", bufs=2, space="PSUM"))
            trps = mctx.enter_context(tc.tile_pool(name="trps", bufs=3, space="PSUM"))
            hdps = mctx.enter_context(tc.tile_pool(name="hdps", bufs=1, space="PSUM"))
            hpool = mctx.enter_context(tc.tile_pool(name="hpool", bufs=2))
            lpool = mctx.enter_context(tc.tile_pool(name="lpool", bufs=2))
            stg = mctx.enter_context(tc.tile_pool(name="stg", bufs=1))
            cmp_ = mctx.enter_context(tc.tile_pool(name="cmp", bufs=1))
            big = mctx.enter_context(tc.tile_pool(name="big", bufs=4))
            outp = mctx.enter_context(tc.tile_pool(name="outp", bufs=2))

            for g in range(NG):
                sstage = stg.tile([128, 4096], F32, tag="sstage")
                rstage = stg.tile([128, 4096], F32, tag="rstage")
                for i4 in range(GC):
                    i = GC * g + i4
                    # ---- builder ----
                    bp = bldps.tile([32, 256], F32, tag="bld")
                    for u in range(4):
                        lt = od7[:, 128 * i + 32 * u:128 * i + 32 * (u + 1)]
                        nc.tensor.matmul(bp[:, 32 * u:32 * (u + 1)], lt,
                                         t_projw[:, 32:64], start=True, stop=True)
                        nc.tensor.matmul(bp[:, 128 + 32 * u:128 + 32 * (u + 1)], lt,
                                         t_projw[:, 0:32], start=True, stop=True)
                    l1 = lpool.tile([64, 128], F32R, tag="l1")
                    nc.vector.tensor_copy(l1[0:32, :], bp[:, 0:128])
                    sh1 = lpool.tile([32, 128], F32R, tag="sh1")
                    nc.vector.tensor_copy(sh1[:], bp[:, 128:256])
                    nc.sync.dma_start(out=l1[32:64, :], in_=sh1[:])
                    bp2 = bldps.tile([32, 256], F32, tag="bld")
                    for u in range(4):
                        lt = od7[:, 128 * i + 32 * u:128 * i + 32 * (u + 1)]
                        nc.tensor.matmul(bp2[:, 32 * u:32 * (u + 1)], lt,
                                         t_projw[:, 96:128], start=True, stop=True)
                        nc.tensor.matmul(bp2[:, 128 + 32 * u:128 + 32 * (u + 1)], lt,
                                         t_projw[:, 64:96], start=True, stop=True)
                    l2 = lpool.tile([64, 128], F32R, tag="l2")
                    nc.vector.tensor_copy(l2[0:32, :], bp2[:, 0:128])
                    sh2 = lpool.tile([32, 128], F32R, tag="sh2")
                    nc.vector.tensor_copy(sh2[:], bp2[:, 128:256])
                    nc.sync.dma_start(out=l2[32:64, :], in_=sh2[:])

                    # ---- trunks + relu evac + heads ----
                    h1 = hpool.tile([128, 4096], BF16, tag="h1")
                    h2 = hpool.tile([128, 4096], BF16, tag="h2")
                    for half in range(2):
                        sps = hdps.tile([128, 512], F32, tag="sps")
                        rps = hdps.tile([128, 512], F32, tag="rps")
                        for bq in range(4):
                            b = 4 * half + bq
                            bs = slice(512 * b, 512 * (b + 1))
                            tp1 = trps.tile([128, 512], F32, tag="tr")
                            nc.tensor.matmul(tp1[:], l1[:], t_sel[:, bs.start:bs.stop],
                                             start=True, stop=True)
                            if b % 2 == 0:
                                nc.scalar.activation(h1[:, bs], tp1[:], AF.Relu)
                            else:
                                nc.vector.tensor_scalar_max(h1[:, bs], tp1[:], 0.0)
                            tp2 = trps.tile([128, 512], F32, tag="tr")
                            nc.tensor.matmul(tp2[:], l2[:], t_sel[:, bs.start:bs.stop],
                                             start=True, stop=True)
                            if b % 2 == 1:
                                nc.scalar.activation(h2[:, bs], tp2[:], AF.Relu)
                            else:
                                nc.vector.tensor_scalar_max(h2[:, bs], tp2[:], 0.0)
                            nc.tensor.matmul(sps[32 * bq:32 * bq + 8, :],
                                             t_wsig[:], h1[:, bs],
                                             start=True, stop=True,
                                             tile_position=(0, 32 * bq))
                            nc.tensor.matmul(rps[32 * bq:32 * bq + 24, :],
                                             t_wrgb[:], h2[:, bs],
                                             start=True, stop=True,
                                             tile_position=(0, 32 * bq))
                        sview_w = sstage[:].rearrange(
                            "p (hf rho i4 t) -> p hf rho i4 t",
                            hf=2, rho=4, i4=GC)[:, half, :, i4, :]
                        rview_w = rstage[:].rearrange(
                            "p (hf rho i4 t) -> p hf rho i4 t",
                            hf=2, rho=4, i4=GC)[:, half, :, i4, :]
                        nc.scalar.activation(
                            sview_w, sps[:].rearrange("p (rho t) -> p rho t", t=T),
                            AF.Copy)
                        nc.scalar.activation(
                            rview_w, rps[:].rearrange("p (rho t) -> p rho t", t=T),
                            AF.Tanh, scale=0.5)

                # ---- scatter ----
                # stage row (32q + colidx), col (i4, half, rho, t)
                # sig colidx = 2u+e ; rgb colidx = 6u+3e+c
                # dest partition = 32u + 16*half + 4q + rho, col (i4, t)
                sig = [cmp_.tile([128, 512], F32, tag=f"sig{e}", name=f"sig{e}") for e in range(2)]
                rgb = [cmp_.tile([128, 512], F32, tag=f"rgb{ec}", name=f"rgb{ec}") for ec in range(6)]
                sview = sstage[:].rearrange(
                    "(q r) (hf rho i4t) -> q r hf rho i4t",
                    q=4, hf=2, rho=4)
                rview = rstage[:].rearrange(
                    "(q r) (hf rho i4t) -> q r hf rho i4t",
                    q=4, hf=2, rho=4)
                for e in range(2):
                    for u in range(4):
                        for half in range(2):
                            po = 32 * u + 16 * half
                            nc.sync.dma_start(
                                out=sig[e][po:po + 16, :],
                                in_=sview[:, 2 * u + e, half])
                for e in range(2):
                    for c in range(3):
                        for u in range(4):
                            for half in range(2):
                                po = 32 * u + 16 * half
                                nc.sync.dma_start(
                                    out=rgb[3 * e + c][po:po + 16, :],
                                    in_=rview[:, 6 * u + 3 * e + c, half])

                # ---- composite ----
                wsup = []
                for e in range(2):
                    p = sig[e]
                    r_ = big.tile([128, 512], F32, tag="big")
                    nc.scalar.activation(r_[:], p[:], AF.Relu)
                    aw = big.tile([128, 512], F32, tag="big")
                    nc.scalar.activation(aw[:], p[:], AF.Abs)
                    nc.scalar.activation(aw[:], aw[:], AF.Exp, scale=-1.0)
                    f = big.tile([128, 512], F32, tag="big")
                    nc.vector.tensor_scalar(out=f[:], in0=aw[:], scalar1=c3,
                                            scalar2=c2, op0=OP.mult, op1=OP.add)
                    nc.vector.tensor_tensor(out=f[:], in0=f[:], in1=aw[:], op=OP.mult)
                    nc.vector.tensor_scalar_add(f[:], f[:], c1)
                    nc.vector.tensor_tensor(out=f[:], in0=f[:], in1=aw[:], op=OP.mult)
                    nc.vector.tensor_scalar_add(f[:], f[:], c0)
                    sp = big.tile([128, 512], F32, tag="big")
                    nc.vector.tensor_tensor(out=sp[:], in0=f[:], in1=r_[:], op=OP.add)
                    E = big.tile([128, 512], F32, tag="big")
                    for i4 in range(GC):
                        i = GC * g + i4
                        cs = slice(T * i4, T * (i4 + 1))
                        nc.scalar.activation(E[:, cs], sp[:, cs], AF.Exp,
                                             scale=ndz[:, i:i + 1])
                        ls = slice(T * (i4 + 1) - 1, T * (i4 + 1))
                        nc.scalar.activation(E[:, ls], sp[:, ls], AF.Exp,
                                             scale=ndz8[:, i:i + 1])
                    omE = big.tile([128, 512], F32, tag="big")
                    nc.vector.tensor_scalar(out=omE[:], in0=E[:], scalar1=-1.0,
                                            scalar2=1.0, op0=OP.mult, op1=OP.add)
                    Ti = big.tile([128, 512], F32, tag="big")
                    w = cmp_.tile([128, 512], F32, tag=f"w{e}")
                    for i4 in range(GC):
                        cs = slice(T * i4, T * (i4 + 1))
                        nc.vector.tensor_tensor_scan(
                            Ti[:, cs], E[:, cs], E[:, cs], 1.0,
                            op0=OP.mult, op1=OP.bypass)
                        nc.vector.tensor_tensor(
                            out=w[:, T * i4 + 1:T * (i4 + 1)],
                            in0=omE[:, T * i4 + 1:T * (i4 + 1)],
                            in1=Ti[:, T * i4:T * (i4 + 1) - 1], op=OP.mult)
                        nc.vector.tensor_copy(w[:, T * i4:T * i4 + 1],
                                              omE[:, T * i4:T * i4 + 1])
                    wsup.append(w)

                # ---- reductions + output ----
                ost = outp.tile([128, 36], F32, tag="ost")
                o3d = ost[:].rearrange("p (i4 c) -> p i4 c", c=9)
                prod = big.tile([128, 512], F32, tag="big")
                wsd = cmp_.tile([128, 8], F32, tag="wsd")
                for e in range(2):
                    w = wsup[e]
                    w3 = w[:].rearrange("p (i4 t) -> p i4 t", t=T)
                    base = 5 * e
                    if e == 0:
                        nc.vector.reduce_sum(o3d[:, :, 4], w3[:], axis=AX.X)
                        ws_ap = o3d[:, :, 4]
                    else:
                        nc.vector.reduce_sum(wsd[:, 0:4], w3[:], axis=AX.X)
                        ws_ap = wsd[:, 0:4]
                    nc.vector.tensor_tensor(out=prod[:], in0=w[:], in1=t_zsup[:],
                                            op=OP.mult)
                    nc.vector.reduce_sum(o3d[:, :, base + 3],
                                         prod[:].rearrange("p (i4 t) -> p i4 t", t=T),
                                         axis=AX.X)
                    for c in range(3):
                        nc.vector.tensor_tensor(out=prod[:], in0=w[:],
                                                in1=rgb[3 * e + c][:], op=OP.mult)
                        red = cmp_.tile([128, 4], F32, tag="red")
                        nc.vector.reduce_sum(red[:],
                                             prod[:].rearrange("p (i4 t) -> p i4 t", t=T),
                                             axis=AX.X)
                        img = cmp_.tile([128, 4], F32, tag="img")
                        nc.vector.tensor_tensor(out=img[:], in0=red[:], in1=ws_ap,
                                                op=OP.subtract)
                        nc.vector.tensor_scalar(out=o3d[:, :, base + c], in0=img[:],
                                                scalar1=0.5, scalar2=1.0,
                                                op0=OP.mult, op1=OP.add)
                nc.sync.dma_start(
                    out=out.rearrange("(i p) c -> p i c", p=128)[:, GC * g:GC * (g + 1), :],
                    in_=o3d[:])

    nc.compile()
    return nc


def _host_consts(W1, b1, Wsig, Wsig_d, Wc1, bc1, Wc2, Wc2_d):
    import ml_dtypes
    z = (np.arange(T, dtype=np.float64) / (T - 1.0))
    if "zrow" not in _CACHE:
        _CACHE["zrow"] = np.tile(z.astype(np.float32), GC).reshape(1, 512)
    pw = np.zeros((10, 128), np.float32)
    # rows: otil(3) ones dtil(3) draw(3) ; cols 0:32 oW1b, 32:64 dW1,
    # 64:96 oWcb (dirs via raw d), 96:128 dWc
    pw[0:3, 0:32] = W1
    pw[3, 0:32] = b1
    pw[4:7, 32:64] = W1
    pw[0:3, 64:96] = Wc1[0:3]
    pw[3, 64:96] = bc1
    pw[7:10, 64:96] = Wc1[3:6]
    pw[4:7, 96:128] = Wc1[0:3]
    wsigb = np.zeros((128, 8), np.float32)
    wrgbb = np.zeros((128, 24), np.float32)
    for u in range(4):
        ro = slice(32 * u, 32 * u + H)
        wsigb[ro, 2 * u + 0] = Wsig[:, 0]
        wsigb[ro, 2 * u + 1] = Wsig_d[:, 0]
        for c in range(3):
            wrgbb[ro, 6 * u + 0 + c] = Wc2[:, c]
            wrgbb[ro, 6 * u + 3 + c] = Wc2_d[:, c]
    return {
        "onesr": np.ones((1, 4096), np.float32),
        "projw": pw,
        "zrow": _CACHE["zrow"],
        "wsigb": wsigb.astype(ml_dtypes.bfloat16),
        "wrgbb": wrgbb.astype(ml_dtypes.bfloat16),
    }


def kernel(rays_o, rays_d, W1, b1, Wsig, Wsig_d, Wc1, bc1, Wc2, Wc2_d,
           num_steps):
    from concourse.bass_utils import run_bass_kernel_spmd

    assert int(num_steps) == T
    if "nc" not in _CACHE:
        _CACHE["nc"] = _build_module()
    nc = _CACHE["nc"]

    d = (rays_d / np.linalg.norm(rays_d, axis=-1, keepdims=True)).astype(np.float32)
    consts = _host_consts(W1, b1, Wsig, Wsig_d, Wc1, bc1, Wc2, Wc2_d)
    in_maps = []
    for cidx in range(N_CORES):
        sl = slice(cidx * PER, (cidx + 1) * PER)
        m = dict(consts)
        m["ob"] = np.ascontiguousarray(rays_o[sl]).ravel()
        m["db"] = np.ascontiguousarray(d[sl]).ravel()
        in_maps.append(m)
    res = run_bass_kernel_spmd(nc, in_maps, core_ids=list(range(N_CORES)))
    return np.concatenate([res.results[c]["out"] for c in range(N_CORES)],
                          axis=0).astype(np.float32)


if __name__ == "__main__":
    rng = np.random.default_rng(0)
    ins = {
        "rays_o": (rng.random((N_RAYS, 3), dtype=np.float32) - 0.5),
        "rays_d": rng.standard_normal((N_RAYS, 3)).astype(np.float32),
        "W1": rng.standard_normal((3, H)).astype(np.float32) * 0.5,
        "b1": np.zeros((H,), np.float32),
        "Wsig": rng.standard_normal((H, 1)).astype(np.float32) * 0.5,
        "Wsig_d": rng.standard_normal((H, 1)).astype(np.float32) * 0.5,
        "Wc1": rng.standard_normal((6, H)).astype(np.float32) * 0.5,
        "bc1": np.zeros((H,), np.float32),
        "Wc2": rng.standard_normal((H, 3)).astype(np.float32) * 0.5,
        "Wc2_d": rng.standard_normal((H, 3)).astype(np.float32) * 0.5,
        "num_steps": 128,
    }
    o = kernel(**ins)
    print("out", o.shape, o.dtype, np.isfinite(o).all())
